# revision 1
# baseline (speedup 1.0000x reference)
"""2-layer GCN (PyG GCNConv x2 + relu + log_softmax) on 8 Trainium2 NeuronCores.

Strategy: shard destination nodes (and their incoming edges) across the 8
cores. Each layer:
  1. dense  h = x @ W  data-parallel over the core's node shard,
     scaled to g = dinv * h  (dinv = 1/sqrt(weighted in-degree + self loop))
  2. AllGather the g-shards into a replicated [N, C] table
  3. edge pass: dma_gather g[src] rows for the core's (dst-sorted, padded)
     edges, build selection matrices S^T[e, n] = w_e * (dst_local[e] == n)
     on the vector engine, segment-sum via TensorE matmuls accumulated in
     PSUM per 128-node destination tile
  4. epilogue out[n] = dinv[n] * (psum[n] + g[n]) + b  (+relu / log_softmax)

dma_gather uses int16 indices, so the node table is split in two halves of
25000 rows; each dst-tile's edge list is partitioned into A-half / B-half
groups, each padded to a multiple of 128 (uniform tile counts KA/KB across
cores and dst-tiles so a single SPMD program works for all cores).
"""
import sys

sys.path.insert(0, "/opt/trn_rl_repo")

import numpy as np

from concourse import bass, mybir, bacc
import concourse.tile as tile
from concourse.bass_utils import run_bass_kernel_spmd

N = 50000
E = 800000
INCH = 128
HID = 128
OUT = 64
NCORES = 8
NSH = N // NCORES          # 6250 nodes per shard
P = 128
NT = (NSH + P - 1) // P    # 49 dst tiles per shard
NPAD = NT * P              # 6272
LOWN = 3200                # low-half padded positions per shard (25 tiles)
HIGHN = 3072               # high-half padded positions per shard (24 tiles)
TRA = NCORES * LOWN        # 25600 rows in table A (int16-safe)
TRB = NCORES * HIGHN       # 24576 rows in table B
CH = 16                    # gather chunk size in edge tiles

f32 = mybir.dt.float32
i16 = mybir.dt.int16
i32 = mybir.dt.int32
AF = mybir.ActivationFunctionType
ALU = mybir.AluOpType

_PROGRAM_CACHE = {}


def _build_program(KA, KB, trace=False, collectives=True, skip=(), with_bias=True):
    key = (KA, KB, collectives, tuple(skip), with_bias)
    if key in _PROGRAM_CACHE:
        return _PROGRAM_CACHE[key]

    TPA = NT * KA            # edge tiles in A region
    TPB = NT * KB
    TP = TPA + TPB
    EPAD = TP * P
    KM = max(KA, KB)

    nc = bacc.Bacc("TRN2", target_bir_lowering=False, debug=False,
                   enable_asserts=True, num_devices=NCORES)

    # inputs
    xT_d = nc.dram_tensor("xT", [P, NPAD], f32, kind="ExternalInput")
    idxw_d = nc.dram_tensor("idxw", [P, EPAD // 16], i16, kind="ExternalInput")
    dstl_d = nc.dram_tensor("dstlT", [P, TP], f32, kind="ExternalInput")
    wt_d = nc.dram_tensor("wT", [P, TP], f32, kind="ExternalInput")
    dinv_d = nc.dram_tensor("dinv", [P, NT], f32, kind="ExternalInput")
    w1_d = nc.dram_tensor("W1", [INCH, HID], f32, kind="ExternalInput")
    w2_d = nc.dram_tensor("W2", [HID, OUT], f32, kind="ExternalInput")
    b1_d = nc.dram_tensor("b1b", [P, HID], f32, kind="ExternalInput")
    b2_d = nc.dram_tensor("b2b", [P, OUT], f32, kind="ExternalInput")
    id_d = nc.dram_tensor("ident", [P, P], f32, kind="ExternalInput")

    out_d = nc.dram_tensor("out", [NPAD, OUT], f32, kind="ExternalOutput")

    # internal DRAM
    g1_d = nc.dram_tensor("g1_shard", [NPAD, HID], f32)
    g2_d = nc.dram_tensor("g2_shard", [NPAD, OUT], f32)
    t1a_d = nc.dram_tensor("table1a", [TRA, HID], f32, addr_space="Shared")
    t1b_d = nc.dram_tensor("table1b", [TRB, HID], f32, addr_space="Shared")
    t2a_d = nc.dram_tensor("table2a", [TRA, OUT], f32, addr_space="Shared")
    t2b_d = nc.dram_tensor("table2b", [TRB, OUT], f32, addr_space="Shared")

    groups = [list(range(NCORES))]

    with tile.TileContext(nc) as tc:
        with (
            tc.tile_pool(name="pers", bufs=1) as pers,
            tc.tile_pool(name="psd", bufs=2, space="PSUM") as psd,
        ):
            # persistent tiles
            idxw = pers.tile([P, EPAD // 16], i16)
            nc.sync.dma_start(idxw[:], idxw_d[:])
            dstlT = pers.tile([P, TP], f32)
            nc.sync.dma_start(dstlT[:], dstl_d[:])
            wT = pers.tile([P, TP], f32)
            nc.sync.dma_start(wT[:], wt_d[:])
            dinv = pers.tile([P, NT], f32)
            nc.sync.dma_start(dinv[:], dinv_d[:])
            w1 = pers.tile([INCH, HID], f32)
            nc.sync.dma_start(w1[:], w1_d[:])
            w2 = pers.tile([HID, OUT], f32)
            nc.sync.dma_start(w2[:], w2_d[:])
            if with_bias:
                b1b = pers.tile([P, HID], f32)
                nc.sync.dma_start(b1b[:], b1_d[:])
                b2b = pers.tile([P, OUT], f32)
                nc.sync.dma_start(b2b[:], b2_d[:])
            ident = pers.tile([P, P], f32)
            nc.sync.dma_start(ident[:], id_d[:])

            iota_i = pers.tile([P, P], i32, tag="iota_i")
            nc.gpsimd.iota(iota_i[:], pattern=[[1, P]], base=0,
                           channel_multiplier=0)
            cols_f = pers.tile([P, P], f32)
            nc.vector.tensor_copy(cols_f[:], iota_i[:])

            g1_sb = pers.tile([P, NT * HID], f32)
            fin_sb = pers.tile([P, NT * OUT], f32)
            relu1T = pers.tile([P, NPAD], f32)
            g2_sb = pers.tile([P, NT * OUT], f32)

            # ---------- phase B: dense layer 1, g1 = dinv * (x @ W1) ----------
            LT = LOWN // P  # 25 low-half tiles
            with tc.tile_pool(name="xb", bufs=1) as xb:
                xT = xb.tile([P, NPAD], f32)
                nc.sync.dma_start(xT[:], xT_d[:])
                for t in range(NT):
                    ps = psd.tile([P, HID], f32, tag="psd")
                    nc.tensor.matmul(ps[:], lhsT=xT[:, t * P:(t + 1) * P],
                                     rhs=w1[:], start=True, stop=True)
                    nc.vector.tensor_scalar(
                        out=g1_sb[:, t * HID:(t + 1) * HID], in0=ps[:],
                        scalar1=dinv[:, t:t + 1], scalar2=None, op0=ALU.mult)
                    if t == LT - 1:
                        # low half complete -> start its AllGather now; it
                        # overlaps the high-half dense compute + B collective
                        nc.sync.dma_start(
                            g1_d[:LOWN].rearrange("(t p) c -> p t c", p=P),
                            g1_sb[:, :LT * HID].rearrange(
                                "p (t c) -> p t c", c=HID))
                        if collectives:
                            nc.gpsimd.collective_compute(
                                "AllGather", ALU.bypass, replica_groups=groups,
                                ins=[g1_d[:LOWN, :]], outs=[t1a_d[:]])
                        else:
                            nc.gpsimd.dma_start(t1a_d[:LOWN, :], g1_d[:LOWN, :])
            nc.sync.dma_start(
                g1_d[LOWN:].rearrange("(t p) c -> p t c", p=P),
                g1_sb[:, LT * HID:].rearrange("p (t c) -> p t c", c=HID))
            if collectives:
                nc.gpsimd.collective_compute(
                    "AllGather", ALU.bypass, replica_groups=groups,
                    ins=[g1_d[LOWN:, :]], outs=[t1b_d[:]])
            else:
                nc.gpsimd.dma_start(t1b_d[:HIGHN, :], g1_d[LOWN:, :])

            # ---------- phase C: edge pass layer 1 ----------
            def edge_pass(tabA_d, tabB_d, C, KA_, KB_, TPA_, epilogue):
                """Gather+SpMM over all dst tiles; epilogue(t, psum_tile)."""
                nchA = (NT * KA_ + CH - 1) // CH
                nchB = (NT * KB_ + CH - 1) // CH
                chunksA = [None] * nchA
                chunksB = [None] * nchB

                with (
                    tc.tile_pool(name="gbufA", bufs=2) as gpA,
                    tc.tile_pool(name="gbufB", bufs=2) as gpB,
                    tc.tile_pool(name="stbuf", bufs=4) as stp,
                    tc.tile_pool(name="pse", bufs=4, space="PSUM") as pse,
                ):
                    def chunk(region, c):
                        lst, pool, tpr, coff = (
                            (chunksA, gpA, NT * KA_, 0) if region == 0 else
                            (chunksB, gpB, NT * KB_, NT * KA_))
                        if lst[c] is None:
                            ct = min(CH, tpr - c * CH)  # tiles in chunk
                            buf = pool.tile([P, CH * C], f32,
                                            tag=f"g{region}")
                            if "gather" in skip:
                                lst[c] = buf
                                return buf
                            half = tabA_d[:] if region == 0 else tabB_d[:]
                            col0 = (coff + c * CH) * (P // 16)
                            nc.gpsimd.dma_gather(
                                out_ap=buf[:, :ct * C].rearrange(
                                    "p (k c) -> p k c", c=C),
                                in_ap=half,
                                idxs_ap=idxw[:, col0:col0 + ct * (P // 16)],
                                num_idxs=ct * P,
                                num_idxs_reg=ct * P,
                                elem_size=C,
                                single_packet=False,
                            )
                            lst[c] = buf
                        return lst[c]

                    for t in range(NT):
                        ps = pse.tile([P, C], f32, tag="pse")
                        # selection matrices for this tile's A and B groups
                        stA = stp.tile([P, KA_ * P], f32, tag="stA")
                        stB = stp.tile([P, KB_ * P], f32, tag="stB")
                        for k in range(KA_):
                            j = t * KA_ + k
                            if "st" in skip:
                                break
                            nc.vector.tensor_scalar(
                                out=stA[:, k * P:(k + 1) * P], in0=cols_f[:],
                                scalar1=dstlT[:, j:j + 1],
                                scalar2=wT[:, j:j + 1],
                                op0=ALU.is_equal, op1=ALU.mult)
                        for k in range(KB_):
                            j = NT * KA_ + t * KB_ + k
                            if "st" in skip:
                                break
                            nc.vector.tensor_scalar(
                                out=stB[:, k * P:(k + 1) * P], in0=cols_f[:],
                                scalar1=dstlT[:, j:j + 1],
                                scalar2=wT[:, j:j + 1],
                                op0=ALU.is_equal, op1=ALU.mult)

                        for k in range(KA_):
                            j = t * KA_ + k
                            buf = chunk(0, j // CH)
                            slot = j % CH
                            if "mm" in skip: continue
                            nc.tensor.matmul(
                                ps[:], lhsT=stA[:, k * P:(k + 1) * P],
                                rhs=buf[:, slot * C:(slot + 1) * C],
                                start=(k == 0),
                                stop=(KB_ == 0 and k == KA_ - 1))
                        for k in range(KB_):
                            j = t * KB_ + k
                            buf = chunk(1, j // CH)
                            slot = j % CH
                            if "mm" in skip: continue
                            nc.tensor.matmul(
                                ps[:], lhsT=stB[:, k * P:(k + 1) * P],
                                rhs=buf[:, slot * C:(slot + 1) * C],
                                start=(KA_ == 0 and k == 0),
                                stop=(k == KB_ - 1))
                        epilogue(t, ps)

            with tc.tile_pool(name="ep1", bufs=3) as ep1, \
                 tc.tile_pool(name="pst", bufs=2, space="PSUM") as pst:
                def epi1(t, ps):
                    nt = min(P, NSH - t * P)
                    tmp = ep1.tile([P, HID], f32, tag="tmp1")
                    nc.vector.tensor_tensor(
                        out=tmp[:], in0=ps[:],
                        in1=g1_sb[:, t * HID:(t + 1) * HID], op=ALU.add)
                    nc.vector.tensor_scalar(
                        out=tmp[:], in0=tmp[:], scalar1=dinv[:, t:t + 1],
                        scalar2=None, op0=ALU.mult)
                    if with_bias:
                        nc.vector.tensor_tensor(
                            out=tmp[:], in0=tmp[:], in1=b1b[:], op=ALU.add)
                    relu = ep1.tile([P, HID], f32, tag="relu1")
                    nc.scalar.activation(relu[:], tmp[:], AF.Relu)
                    pt = pst.tile([P, P], f32, tag="pst")
                    nc.tensor.transpose(pt[:], relu[:], ident[:])
                    nc.vector.tensor_copy(relu1T[:, t * P:(t + 1) * P], pt[:])

                edge_pass(t1a_d, t1b_d, HID, KA, KB, TPA, epi1)

            # ---------- phase D: dense layer 2 ----------
            for t in range(NT):
                ps = psd.tile([P, OUT], f32, tag="psd")
                nc.tensor.matmul(ps[:], lhsT=relu1T[:, t * P:(t + 1) * P],
                                 rhs=w2[:], start=True, stop=True)
                nc.vector.tensor_scalar(
                    out=g2_sb[:, t * OUT:(t + 1) * OUT], in0=ps[:],
                    scalar1=dinv[:, t:t + 1], scalar2=None, op0=ALU.mult)
                if t == LT - 1:
                    nc.sync.dma_start(
                        g2_d[:LOWN].rearrange("(t p) c -> p t c", p=P),
                        g2_sb[:, :LT * OUT].rearrange(
                            "p (t c) -> p t c", c=OUT))
                    if collectives:
                        nc.gpsimd.collective_compute(
                            "AllGather", ALU.bypass, replica_groups=groups,
                            ins=[g2_d[:LOWN, :]], outs=[t2a_d[:]])
                    else:
                        nc.gpsimd.dma_start(t2a_d[:LOWN, :], g2_d[:LOWN, :])
            nc.sync.dma_start(
                g2_d[LOWN:].rearrange("(t p) c -> p t c", p=P),
                g2_sb[:, LT * OUT:].rearrange("p (t c) -> p t c", c=OUT))
            if collectives:
                nc.gpsimd.collective_compute(
                    "AllGather", ALU.bypass, replica_groups=groups,
                    ins=[g2_d[LOWN:, :]], outs=[t2b_d[:]])
            else:
                nc.gpsimd.dma_start(t2b_d[:HIGHN, :], g2_d[LOWN:, :])

            # ---------- phase E: edge pass layer 2 + log_softmax ----------
            # Ln is batched over all tiles at the end (one act-table load
            # instead of alternating Exp/Ln per tile).
            o_sb = pers.tile([P, NT * OUT], f32)
            negm_sb = pers.tile([P, NT], f32)
            se_sb = pers.tile([P, NT], f32)
            lse_sb = pers.tile([P, NT], f32)
            with tc.tile_pool(name="ep2", bufs=3) as ep2:
                def epi2(t, ps):
                    osl = o_sb[:, t * OUT:(t + 1) * OUT]
                    nc.vector.tensor_tensor(
                        out=osl, in0=ps[:],
                        in1=g2_sb[:, t * OUT:(t + 1) * OUT], op=ALU.add)
                    nc.vector.tensor_scalar(
                        out=osl, in0=osl, scalar1=dinv[:, t:t + 1],
                        scalar2=None, op0=ALU.mult)
                    if with_bias:
                        nc.vector.tensor_tensor(
                            out=osl, in0=osl, in1=b2b[:], op=ALU.add)
                    m = ep2.tile([P, 1], f32, tag="m")
                    nc.vector.tensor_reduce(
                        out=m[:], in_=osl, axis=mybir.AxisListType.X,
                        op=ALU.max)
                    nc.vector.tensor_scalar(
                        out=negm_sb[:, t:t + 1], in0=m[:], scalar1=-1.0,
                        scalar2=None, op0=ALU.mult)
                    ex = ep2.tile([P, OUT], f32, tag="ex")
                    nc.scalar.activation(ex[:], osl, AF.Exp,
                                         bias=negm_sb[:, t:t + 1],
                                         accum_out=se_sb[:, t:t + 1])

                edge_pass(t2a_d, t2b_d, OUT, KA, KB, TPA, epi2)
                nc.scalar.activation(lse_sb[:], se_sb[:], AF.Ln)
                for t in range(NT):
                    nc.vector.tensor_scalar(
                        out=fin_sb[:, t * OUT:(t + 1) * OUT],
                        in0=o_sb[:, t * OUT:(t + 1) * OUT],
                        scalar1=negm_sb[:, t:t + 1],
                        scalar2=lse_sb[:, t:t + 1],
                        op0=ALU.add, op1=ALU.subtract)
                nc.sync.dma_start(
                    out_d[:].rearrange("(t p) c -> p t c", p=P),
                    fin_sb[:].rearrange("p (t c) -> p t c", c=OUT))

    nc.compile()
    _PROGRAM_CACHE[key] = nc
    return nc


def _wrap_idx(lidx):
    """[EPAD] int -> [128, EPAD//16] int16 (16-partition wrap, 8x replicated)."""
    n = lidx.shape[0]
    w16 = lidx.reshape(n // 16, 16).T.astype(np.int16)   # [16, n/16]
    return np.ascontiguousarray(np.tile(w16, (8, 1)))


def _prep_inputs(x, edge_index, edge_weight):
    src = np.asarray(edge_index[0], dtype=np.int64)
    dst = np.asarray(edge_index[1], dtype=np.int64)
    w = np.asarray(edge_weight, dtype=np.float32)
    x = np.asarray(x, dtype=np.float32)

    deg = np.bincount(dst, weights=w.astype(np.float64), minlength=N)
    deg = deg.astype(np.float32) + 1.0
    dinv = (1.0 / np.sqrt(deg)).astype(np.float32)

    shard_src = src // NSH
    shard_dst = dst // NSH
    # half = src's ORIGINAL local id >= LOWN; the balancer below preserves
    # half membership (low nodes only land in permuted positions < LOWN),
    # so this is permutation-independent.
    halfe = ((src % NSH) >= LOWN).astype(np.int64)

    # Load-balancing permutation per shard: assign nodes to dst tiles so the
    # per-(tile, half) in-edge counts are even, shrinking the uniform KA/KB
    # padding. half(edge) = src_shard >= 4 is permutation-independent.
    perms = []
    iperms = np.empty((NCORES, NSH), np.int64)
    for s in range(NCORES):
        m = shard_dst == s
        dl = dst[m] - s * NSH
        he = halfe[m]
        aA = np.bincount(dl[he == 0], minlength=NSH).astype(np.int64)
        aB = np.bincount(dl[he == 1], minlength=NSH).astype(np.int64)
        tot = aA + aB
        LT = LOWN // P
        cap = np.full(NT, P, np.int64)
        cap[NT - 1] = NSH - (NT - 1) * P     # 106
        load = np.zeros(NT)
        buckets = [[] for _ in range(NT)]
        # low nodes (orig local < LOWN) -> tiles [0, LT); high -> [LT, NT)
        for lo, hi, rng in ((0, LOWN, range(0, LT)),
                            (LOWN, NSH, range(LT, NT))):
            nodes = np.arange(lo, hi)
            order = nodes[np.argsort(-tot[lo:hi], kind="stable")]
            tiles = np.fromiter(rng, np.int64)
            for l in order:
                cand = np.where(cap[tiles] > 0, load[tiles], np.inf)
                tsel = int(tiles[int(np.argmin(cand))])
                buckets[tsel].append(l)
                cap[tsel] -= 1
                load[tsel] += tot[l]
        perm = np.concatenate([np.asarray(b, np.int64) for b in buckets])
        perms.append(perm)
        iperms[s][perm] = np.arange(NSH)
    _prep_inputs.last_perms = perms

    per_core = []
    for s in range(NCORES):
        m = shard_dst == s
        es = src[m]
        ew = w[m]
        half = halfe[m]
        edp = iperms[s][dst[m] - s * NSH]      # permuted local dst
        t = edp >> 7
        pos = iperms[shard_src[m], es % NSH]   # permuted position of src
        # table row within its half-table (A: pos<LOWN, B: pos>=LOWN)
        trow = np.where(pos < LOWN,
                        shard_src[m] * LOWN + pos,
                        shard_src[m] * HIGHN + (pos - LOWN))
        key = (half * NT + t)
        order = np.argsort(key, kind="stable")
        per_core.append((trow[order], edp[order], ew[order],
                         t[order], half[order]))

    # group counts -> uniform KA/KB
    cntA = np.zeros((NCORES, NT), np.int64)
    cntB = np.zeros((NCORES, NT), np.int64)
    for s in range(NCORES):
        es, ed, ew, t, half = per_core[s]
        for h, cnt in ((0, cntA), (1, cntB)):
            hm = half == h
            cnt[s] = np.bincount(t[hm], minlength=NT)
    KA = int(np.ceil(cntA.max() / P))
    KB = int(np.ceil(cntB.max() / P))

    in_maps = []
    for s in range(NCORES):
        es, ed, ew, t, half = per_core[s]
        lidx_all = np.zeros((NT * (KA + KB), P), np.int64)
        dstl_all = np.zeros((NT * (KA + KB), P), np.float32)
        w_all = np.zeros((NT * (KA + KB), P), np.float32)
        pos = 0
        for h, K, base_tile in ((0, KA, 0), (1, KB, NT * KA)):
            hm = half == h
            eh, edh, ewh, th = es[hm], ed[hm], ew[hm], t[hm]
            for tt in range(NT):
                tm = th == tt
                cnt = int(tm.sum())
                row = base_tile + tt * K
                li = eh[tm]
                dl = (edh[tm] & 127).astype(np.float32)
                wv = ewh[tm]
                flat_l = np.zeros(K * P, np.int64)
                flat_d = np.zeros(K * P, np.float32)
                flat_w = np.zeros(K * P, np.float32)
                flat_l[:cnt] = li
                flat_d[:cnt] = dl
                flat_w[:cnt] = wv
                lidx_all[row:row + K] = flat_l.reshape(K, P)
                dstl_all[row:row + K] = flat_d.reshape(K, P)
                w_all[row:row + K] = flat_w.reshape(K, P)

        xs = x[s * NSH + perms[s]]
        xT = np.zeros((P, NPAD), np.float32)
        xT[:, :NSH] = xs.T
        full = np.ones(NPAD, np.float32)
        full[:NSH] = dinv[s * NSH + perms[s]]
        dv = np.ascontiguousarray(full.reshape(NT, P).T)

        in_maps.append({
            "xT": xT,
            "idxw": _wrap_idx(lidx_all.reshape(-1)),
            "dstlT": np.ascontiguousarray(dstl_all.T),
            "wT": np.ascontiguousarray(w_all.T),
            "dinv": dv,
        })
    return in_maps, KA, KB


def kernel(x, edge_index, edge_weight, W1, b1, W2, b2, trace=False):
    in_maps, KA, KB = _prep_inputs(x, edge_index, edge_weight)
    shared = {
        "W1": np.asarray(W1, np.float32),
        "W2": np.asarray(W2, np.float32),
        "b1b": np.tile(np.asarray(b1, np.float32)[None, :], (P, 1)),
        "b2b": np.tile(np.asarray(b2, np.float32)[None, :], (P, 1)),
        "ident": np.eye(P, dtype=np.float32),
    }
    for im in in_maps:
        im.update(shared)

    with_bias = bool(np.any(shared["b1b"]) or np.any(shared["b2b"]))
    nc = _build_program(KA, KB, with_bias=with_bias)
    res = run_bass_kernel_spmd(nc, in_maps, core_ids=list(range(NCORES)),
                               trace=trace)
    perms = _prep_inputs.last_perms
    out = np.empty((N, OUT), np.float32)
    for s in range(NCORES):
        out[s * NSH + perms[s]] = res.results[s]["out"][:NSH]
    kernel.last_results = res
    return out



# revision 4
# speedup vs baseline: 1.1843x; 1.1843x over previous
"""2-layer GCN (PyG GCNConv x2 + relu + log_softmax) on 8 Trainium2 NeuronCores.

Strategy: shard destination nodes (and their incoming edges) across the 8
cores. Each layer:
  1. dense  h = x @ W  data-parallel over the core's node shard,
     scaled to g = dinv * h  (dinv = 1/sqrt(weighted in-degree + self loop))
  2. AllGather the g-shards into a replicated bf16 [N, C] table
  3. edge pass: dma_gather g[src] rows (256B bf16) for the core's
     (dst-sorted, padded) edges, build bf16 selection matrices
     S^T[e, n] = w_e * (dst_local[e] == n) on the vector engine,
     segment-sum via bf16 TensorE matmuls accumulated in PSUM per
     128-node destination tile
  4. epilogue out[n] = dinv[n] * (psum[n] + g[n]) + b  (+relu / log_softmax)

dma_gather uses int16 indices, so the node table is split in two halves
(A: 25600 rows, B: 24576 rows); each dst-tile's edge list is partitioned
into A-half / B-half groups, each padded to a multiple of 128 (uniform tile
counts KA/KB across cores and dst-tiles so a single SPMD program works for
all cores). Tables and outputs use a partition-major row order
(row = p*tiles + t) so every bulk DMA moves contiguous per-partition spans.
Layer 2's 64-channel rows are zero-padded to 128 bf16 channels to satisfy
the 256B-per-descriptor gather minimum.
"""
import sys

sys.path.insert(0, "/opt/trn_rl_repo")

import numpy as np
import ml_dtypes

from concourse import bass, mybir, bacc
import concourse.tile as tile
from concourse.bass_utils import run_bass_kernel_spmd

N = 50000
E = 800000
INCH = 128
HID = 128
OUT = 64
NCORES = 8
NSH = N // NCORES          # 6250 nodes per shard
P = 128
NT = (NSH + P - 1) // P    # 49 dst tiles per shard
NPAD = NT * P              # 6272
LT = 25                    # low-half tiles
HT = NT - LT               # 24 high-half tiles
LOWN = LT * P              # 3200 low-half padded positions per shard
HIGHN = HT * P             # 3072 high-half padded positions per shard
TRA = NCORES * LOWN        # 25600 rows in table A (int16-safe)
TRB = NCORES * HIGHN       # 24576 rows in table B
CH = 32                    # gather chunk size in edge tiles

f32 = mybir.dt.float32
bf16 = mybir.dt.bfloat16
i16 = mybir.dt.int16
i32 = mybir.dt.int32
AF = mybir.ActivationFunctionType
ALU = mybir.AluOpType

npbf16 = ml_dtypes.bfloat16

_PROGRAM_CACHE = {}


def _build_program(KA, KB, trace=False, collectives=True, skip=(), with_bias=True):
    key = (KA, KB, collectives, tuple(skip), with_bias)
    if key in _PROGRAM_CACHE:
        return _PROGRAM_CACHE[key]

    TPA = NT * KA            # edge tiles in A region
    TPB = NT * KB
    TP = TPA + TPB
    EPAD = TP * P

    nc = bacc.Bacc("TRN2", target_bir_lowering=False, debug=False,
                   enable_asserts=True, num_devices=NCORES)

    # inputs
    xT_d = nc.dram_tensor("xT", [P, NPAD], bf16, kind="ExternalInput")
    idxw_d = nc.dram_tensor("idxw", [P, EPAD // 16], i16, kind="ExternalInput")
    dstl_d = nc.dram_tensor("dstlT", [P, TP], f32, kind="ExternalInput")
    wt_d = nc.dram_tensor("wT", [P, TP], f32, kind="ExternalInput")
    dinv_d = nc.dram_tensor("dinv", [P, NT], f32, kind="ExternalInput")
    w1_d = nc.dram_tensor("W1", [INCH, HID], bf16, kind="ExternalInput")
    w2_d = nc.dram_tensor("W2", [HID, OUT], bf16, kind="ExternalInput")
    b1_d = nc.dram_tensor("b1b", [P, HID], f32, kind="ExternalInput")
    b2_d = nc.dram_tensor("b2b", [P, OUT], f32, kind="ExternalInput")
    id_d = nc.dram_tensor("ident", [P, P], bf16, kind="ExternalInput")

    out_d = nc.dram_tensor("out", [P, NT * OUT], f32, kind="ExternalOutput")

    # internal DRAM (partition-major row order: row = p*LT + t  /  p*HT + t')
    g1a_d = nc.dram_tensor("g1a_shard", [LOWN, HID], bf16)
    g1b_d = nc.dram_tensor("g1b_shard", [HIGHN, HID], bf16)
    g2a_d = nc.dram_tensor("g2a_shard", [LOWN, P], bf16)
    g2b_d = nc.dram_tensor("g2b_shard", [HIGHN, P], bf16)
    t1a_d = nc.dram_tensor("table1a", [TRA, HID], bf16, addr_space="Shared")
    t1b_d = nc.dram_tensor("table1b", [TRB, HID], bf16, addr_space="Shared")
    t2a_d = nc.dram_tensor("table2a", [TRA, P], bf16, addr_space="Shared")
    t2b_d = nc.dram_tensor("table2b", [TRB, P], bf16, addr_space="Shared")

    groups = [list(range(NCORES))]

    with tile.TileContext(nc) as tc:
        with (
            tc.tile_pool(name="pers", bufs=1) as pers,
            tc.tile_pool(name="psd", bufs=2, space="PSUM") as psd,
        ):
            # persistent tiles
            idxw = pers.tile([P, EPAD // 16], i16)
            nc.sync.dma_start(idxw[:], idxw_d[:])
            dstlT = pers.tile([P, TP], f32)
            nc.sync.dma_start(dstlT[:], dstl_d[:])
            wT = pers.tile([P, TP], f32)
            nc.sync.dma_start(wT[:], wt_d[:])
            dinv = pers.tile([P, NT], f32)
            nc.sync.dma_start(dinv[:], dinv_d[:])
            w1 = pers.tile([INCH, HID], bf16)
            nc.sync.dma_start(w1[:], w1_d[:])
            w2 = pers.tile([HID, OUT], bf16)
            nc.sync.dma_start(w2[:], w2_d[:])
            if with_bias:
                b1b = pers.tile([P, HID], f32)
                nc.sync.dma_start(b1b[:], b1_d[:])
                b2b = pers.tile([P, OUT], f32)
                nc.sync.dma_start(b2b[:], b2_d[:])
            ident = pers.tile([P, P], bf16)
            nc.sync.dma_start(ident[:], id_d[:])

            iota_i = pers.tile([P, P], i32, tag="iota_i")
            nc.gpsimd.iota(iota_i[:], pattern=[[1, P]], base=0,
                           channel_multiplier=0)
            cols_b = pers.tile([P, P], bf16)
            nc.vector.tensor_copy(cols_b[:], iota_i[:])

            g1_sb = pers.tile([P, NT * HID], bf16)
            relu1T = pers.tile([P, NPAD], bf16)
            # g2 staged zero-padded to 128 channels (real 64 in low half of
            # each 128-col block)
            g2_sb = pers.tile([P, NT * P], bf16)
            nc.vector.memset(g2_sb[:], 0.0)
            o_sb = pers.tile([P, NT * OUT], f32)
            negm_sb = pers.tile([P, NT], f32)
            se_sb = pers.tile([P, NT], f32)
            lse_sb = pers.tile([P, NT], f32)

            # ---------- phase B: dense layer 1, g1 = dinv * (x @ W1) ----------
            with tc.tile_pool(name="xb", bufs=1) as xb:
                xT = xb.tile([P, NPAD], bf16)
                nc.sync.dma_start(xT[:], xT_d[:])
                for t in range(NT):
                    ps = psd.tile([P, HID], f32, tag="psd")
                    nc.tensor.matmul(ps[:], lhsT=xT[:, t * P:(t + 1) * P],
                                     rhs=w1[:], start=True, stop=True)
                    nc.vector.tensor_scalar(
                        out=g1_sb[:, t * HID:(t + 1) * HID], in0=ps[:],
                        scalar1=dinv[:, t:t + 1], scalar2=None, op0=ALU.mult)
                    if t == LT - 1:
                        # low half complete -> start its AllGather now; it
                        # overlaps the high-half dense compute + B collective
                        nc.sync.dma_start(
                            g1a_d[:].rearrange("(p t) c -> p t c", p=P),
                            g1_sb[:, :LT * HID].rearrange(
                                "p (t c) -> p t c", c=HID))
                        if collectives:
                            nc.gpsimd.collective_compute(
                                "AllGather", ALU.bypass, replica_groups=groups,
                                ins=[g1a_d[:]], outs=[t1a_d[:]])
                        else:
                            nc.gpsimd.dma_start(t1a_d[:LOWN, :], g1a_d[:])
            nc.sync.dma_start(
                g1b_d[:].rearrange("(p t) c -> p t c", p=P),
                g1_sb[:, LT * HID:].rearrange("p (t c) -> p t c", c=HID))
            if collectives:
                nc.gpsimd.collective_compute(
                    "AllGather", ALU.bypass, replica_groups=groups,
                    ins=[g1b_d[:]], outs=[t1b_d[:]])
            else:
                nc.gpsimd.dma_start(t1b_d[:HIGHN, :], g1b_d[:])

            # ---------- edge pass over all dst tiles ----------
            def edge_pass(tabA_d, tabB_d, RW, epilogue):
                """Gather+SpMM over all dst tiles; epilogue(t, psum_tile).

                Gathered rows are always 128 bf16 channels (256B descriptors);
                the matmul consumes the first RW of them.
                """
                nchA = (NT * KA + CH - 1) // CH
                nchB = (NT * KB + CH - 1) // CH
                chunksA = [None] * nchA
                chunksB = [None] * nchB

                with (
                    tc.tile_pool(name="gbufA", bufs=2) as gpA,
                    tc.tile_pool(name="gbufB", bufs=2) as gpB,
                    tc.tile_pool(name="stbuf", bufs=4) as stp,
                    tc.tile_pool(name="pse", bufs=2, space="PSUM") as pse,
                ):
                    def chunk(region, c):
                        lst, pool, tpr, coff = (
                            (chunksA, gpA, NT * KA, 0) if region == 0 else
                            (chunksB, gpB, NT * KB, NT * KA))
                        if lst[c] is None:
                            ct = min(CH, tpr - c * CH)  # tiles in chunk
                            buf = pool.tile([P, CH * P], bf16,
                                            tag=f"g{region}")
                            if "gather" in skip:
                                lst[c] = buf
                                return buf
                            half = tabA_d[:] if region == 0 else tabB_d[:]
                            col0 = (coff + c * CH) * (P // 16)
                            nc.gpsimd.dma_gather(
                                out_ap=buf[:, :ct * P].rearrange(
                                    "p (k c) -> p k c", c=P),
                                in_ap=half,
                                idxs_ap=idxw[:, col0:col0 + ct * (P // 16)],
                                num_idxs=ct * P,
                                num_idxs_reg=ct * P,
                                elem_size=P,
                                single_packet=False,
                            )
                            lst[c] = buf
                        return lst[c]

                    for t in range(NT):
                        ps = pse.tile([P, RW], f32, tag="pse")
                        # selection matrices for this tile's A and B groups
                        stA = stp.tile([P, KA * P], bf16, tag="stA")
                        stB = stp.tile([P, KB * P], bf16, tag="stB")
                        for k in range(KA):
                            j = t * KA + k
                            if "st" in skip:
                                break
                            nc.vector.tensor_scalar(
                                out=stA[:, k * P:(k + 1) * P], in0=cols_b[:],
                                scalar1=dstlT[:, j:j + 1],
                                scalar2=wT[:, j:j + 1],
                                op0=ALU.is_equal, op1=ALU.mult)
                        for k in range(KB):
                            j = NT * KA + t * KB + k
                            if "st" in skip:
                                break
                            nc.vector.tensor_scalar(
                                out=stB[:, k * P:(k + 1) * P], in0=cols_b[:],
                                scalar1=dstlT[:, j:j + 1],
                                scalar2=wT[:, j:j + 1],
                                op0=ALU.is_equal, op1=ALU.mult)

                        for k in range(KA):
                            j = t * KA + k
                            buf = chunk(0, j // CH)
                            slot = j % CH
                            if "mm" in skip: continue
                            nc.tensor.matmul(
                                ps[:], lhsT=stA[:, k * P:(k + 1) * P],
                                rhs=buf[:, slot * P:slot * P + RW],
                                start=(k == 0),
                                stop=(KB == 0 and k == KA - 1))
                        for k in range(KB):
                            j = t * KB + k
                            buf = chunk(1, j // CH)
                            slot = j % CH
                            if "mm" in skip: continue
                            nc.tensor.matmul(
                                ps[:], lhsT=stB[:, k * P:(k + 1) * P],
                                rhs=buf[:, slot * P:slot * P + RW],
                                start=(KA == 0 and k == 0),
                                stop=(k == KB - 1))
                        epilogue(t, ps)

            # ---------- phase C: edge pass layer 1 (+ fused dense layer 2) ----
            with tc.tile_pool(name="ep1", bufs=3) as ep1, \
                 tc.tile_pool(name="pst", bufs=2, space="PSUM") as pst:
                def epi1(t, ps):
                    tmp = ep1.tile([P, HID], f32, tag="tmp1")
                    nc.vector.tensor_tensor(
                        out=tmp[:], in0=ps[:],
                        in1=g1_sb[:, t * HID:(t + 1) * HID], op=ALU.add)
                    nc.vector.tensor_scalar(
                        out=tmp[:], in0=tmp[:], scalar1=dinv[:, t:t + 1],
                        scalar2=None, op0=ALU.mult)
                    if with_bias:
                        nc.vector.tensor_tensor(
                            out=tmp[:], in0=tmp[:], in1=b1b[:], op=ALU.add)
                    relu = ep1.tile([P, HID], bf16, tag="relu1")
                    nc.scalar.activation(relu[:], tmp[:], AF.Relu)
                    pt = pst.tile([P, P], bf16, tag="pst")
                    nc.tensor.transpose(pt[:], relu[:], ident[:])
                    nc.vector.tensor_copy(relu1T[:, t * P:(t + 1) * P], pt[:])

                    # fused dense layer 2 for this tile:
                    # g2 = dinv * (relu1 @ W2), staged zero-padded to 128 ch
                    ps2 = psd.tile([P, OUT], f32, tag="psd2")
                    nc.tensor.matmul(ps2[:], lhsT=relu1T[:, t * P:(t + 1) * P],
                                     rhs=w2[:], start=True, stop=True)
                    nc.vector.tensor_scalar(
                        out=g2_sb[:, t * P:t * P + OUT], in0=ps2[:],
                        scalar1=dinv[:, t:t + 1], scalar2=None, op0=ALU.mult)
                    if t == LT - 1:
                        nc.sync.dma_start(
                            g2a_d[:].rearrange("(p t) c -> p t c", p=P),
                            g2_sb[:, :LT * P].rearrange("p (t c) -> p t c",
                                                        c=P))
                        if collectives:
                            nc.gpsimd.collective_compute(
                                "AllGather", ALU.bypass, replica_groups=groups,
                                ins=[g2a_d[:]], outs=[t2a_d[:]])
                        else:
                            nc.gpsimd.dma_start(t2a_d[:LOWN, :], g2a_d[:])
                    if t == NT - 1:
                        nc.sync.dma_start(
                            g2b_d[:].rearrange("(p t) c -> p t c", p=P),
                            g2_sb[:, LT * P:].rearrange("p (t c) -> p t c",
                                                        c=P))
                        if collectives:
                            nc.gpsimd.collective_compute(
                                "AllGather", ALU.bypass, replica_groups=groups,
                                ins=[g2b_d[:]], outs=[t2b_d[:]])
                        else:
                            nc.gpsimd.dma_start(t2b_d[:HIGHN, :], g2b_d[:])

                edge_pass(t1a_d, t1b_d, HID, epi1)

            # ---------- phase E: edge pass layer 2 + log_softmax ----------
            # Ln is batched over all tiles at the end (one act-table load
            # instead of alternating Exp/Ln per tile).
            with tc.tile_pool(name="ep2", bufs=3) as ep2:
                def epi2(t, ps):
                    osl = o_sb[:, t * OUT:(t + 1) * OUT]
                    nc.vector.tensor_tensor(
                        out=osl, in0=ps[:],
                        in1=g2_sb[:, t * P:t * P + OUT], op=ALU.add)
                    nc.vector.tensor_scalar(
                        out=osl, in0=osl, scalar1=dinv[:, t:t + 1],
                        scalar2=None, op0=ALU.mult)
                    if with_bias:
                        nc.vector.tensor_tensor(
                            out=osl, in0=osl, in1=b2b[:], op=ALU.add)
                    m = ep2.tile([P, 1], f32, tag="m")
                    nc.vector.tensor_reduce(
                        out=m[:], in_=osl, axis=mybir.AxisListType.X,
                        op=ALU.max)
                    nc.vector.tensor_scalar(
                        out=negm_sb[:, t:t + 1], in0=m[:], scalar1=-1.0,
                        scalar2=None, op0=ALU.mult)
                    ex = ep2.tile([P, OUT], f32, tag="ex")
                    nc.scalar.activation(ex[:], osl, AF.Exp,
                                         bias=negm_sb[:, t:t + 1],
                                         accum_out=se_sb[:, t:t + 1])

                edge_pass(t2a_d, t2b_d, OUT, epi2)
                nc.scalar.activation(lse_sb[:], se_sb[:], AF.Ln)
                for t in range(NT):
                    nc.vector.tensor_scalar(
                        out=o_sb[:, t * OUT:(t + 1) * OUT],
                        in0=o_sb[:, t * OUT:(t + 1) * OUT],
                        scalar1=negm_sb[:, t:t + 1],
                        scalar2=lse_sb[:, t:t + 1],
                        op0=ALU.add, op1=ALU.subtract)
                nc.sync.dma_start(out_d[:], o_sb[:])

    nc.compile()
    _PROGRAM_CACHE[key] = nc
    return nc


def _wrap_idx(lidx):
    """[EPAD] int -> [128, EPAD//16] int16 (16-partition wrap, 8x replicated)."""
    n = lidx.shape[0]
    w16 = lidx.reshape(n // 16, 16).T.astype(np.int16)   # [16, n/16]
    return np.ascontiguousarray(np.tile(w16, (8, 1)))


def _balance(aA, aB):
    """Assign a shard's nodes to tiles (low nodes -> tiles [0,LT), high ->
    [LT,NT)) so that per-tile A-half and B-half in-edge counts stay under
    KA*128 / KB*128 with KA=9, KB=8 if possible.  Returns perm (node order:
    position t*128+p holds node perm[t*128+p])."""
    tot = aA + aB
    capA_t = 9 * P    # soft targets
    capB_t = 8 * P
    buckets = [[] for _ in range(NT)]
    for lo, hi, t0, t1 in ((0, LOWN, 0, LT), (LOWN, NSH, LT, NT)):
        nodes = np.arange(lo, hi)
        order = nodes[np.argsort(-tot[lo:hi], kind="stable")]
        tiles = np.arange(t0, t1)
        ntile = t1 - t0
        cap = np.full(ntile, P, np.int64)
        if t1 == NT:
            cap[ntile - 1] = NSH - lo - (ntile - 1) * P
        loadA = np.zeros(ntile)
        loadB = np.zeros(ntile)
        for nd in order:
            a, b = aA[nd], aB[nd]
            score = np.maximum((loadA + a) / capA_t, (loadB + b) / capB_t)
            score = np.where(cap > 0, score, np.inf)
            ti = int(np.argmin(score))
            buckets[tiles[ti]].append(nd)
            cap[ti] -= 1
            loadA[ti] += a
            loadB[ti] += b
    return np.concatenate([np.asarray(b, np.int64) for b in buckets])


def _prep_inputs(x, edge_index, edge_weight):
    src = np.asarray(edge_index[0], dtype=np.int64)
    dst = np.asarray(edge_index[1], dtype=np.int64)
    w = np.asarray(edge_weight, dtype=np.float32)
    x = np.asarray(x, dtype=np.float32)

    deg = np.bincount(dst, weights=w.astype(np.float64), minlength=N)
    deg = deg.astype(np.float32) + 1.0
    dinv = (1.0 / np.sqrt(deg)).astype(np.float32)

    shard_src = src // NSH
    shard_dst = dst // NSH
    # half = src's ORIGINAL local id >= LOWN; the balancer below preserves
    # half membership (low nodes only land in permuted positions < LOWN),
    # so this is permutation-independent.
    halfe = ((src % NSH) >= LOWN).astype(np.int64)

    perms = []
    iperms = np.empty((NCORES, NSH), np.int64)
    for s in range(NCORES):
        m = shard_dst == s
        dl = dst[m] - s * NSH
        he = halfe[m]
        aA = np.bincount(dl[he == 0], minlength=NSH).astype(np.int64)
        aB = np.bincount(dl[he == 1], minlength=NSH).astype(np.int64)
        perm = _balance(aA, aB)
        perms.append(perm)
        iperms[s][perm] = np.arange(NSH)
    _prep_inputs.last_perms = perms

    per_core = []
    for s in range(NCORES):
        m = shard_dst == s
        es = src[m]
        ew = w[m]
        half = halfe[m]
        edp = iperms[s][dst[m] - s * NSH]      # permuted local dst position
        t = edp >> 7
        pos = iperms[shard_src[m], es % NSH]   # permuted position of src
        pt = pos >> 7
        pp = pos & 127
        # table row within its half-table, partition-major:
        # A: row = p*LT + t (t<LT);  B: row = p*HT + (t-LT)
        trow = np.where(pos < LOWN,
                        shard_src[m] * LOWN + pp * LT + pt,
                        shard_src[m] * HIGHN + pp * HT + (pt - LT))
        key = (half * NT + t)
        order = np.argsort(key, kind="stable")
        per_core.append((trow[order], edp[order], ew[order],
                         t[order], half[order]))

    # group counts -> uniform KA/KB
    cntA = np.zeros((NCORES, NT), np.int64)
    cntB = np.zeros((NCORES, NT), np.int64)
    for s in range(NCORES):
        es, ed, ew, t, half = per_core[s]
        for h, cnt in ((0, cntA), (1, cntB)):
            hm = half == h
            cnt[s] = np.bincount(t[hm], minlength=NT)
    KA = int(np.ceil(cntA.max() / P))
    KB = int(np.ceil(cntB.max() / P))

    in_maps = []
    for s in range(NCORES):
        es, ed, ew, t, half = per_core[s]
        lidx_all = np.zeros((NT * (KA + KB), P), np.int64)
        dstl_all = np.zeros((NT * (KA + KB), P), np.float32)
        w_all = np.zeros((NT * (KA + KB), P), np.float32)
        for h, K, base_tile in ((0, KA, 0), (1, KB, NT * KA)):
            hm = half == h
            eh, edh, ewh, th = es[hm], ed[hm], ew[hm], t[hm]
            for tt in range(NT):
                tm = th == tt
                cnt = int(tm.sum())
                row = base_tile + tt * K
                flat_l = np.zeros(K * P, np.int64)
                flat_d = np.zeros(K * P, np.float32)
                flat_w = np.zeros(K * P, np.float32)
                flat_l[:cnt] = eh[tm]
                flat_d[:cnt] = (edh[tm] & 127).astype(np.float32)
                flat_w[:cnt] = ewh[tm]
                lidx_all[row:row + K] = flat_l.reshape(K, P)
                dstl_all[row:row + K] = flat_d.reshape(K, P)
                w_all[row:row + K] = flat_w.reshape(K, P)

        xs = x[s * NSH + perms[s]]
        xT = np.zeros((P, NPAD), np.float32)
        xT[:, :NSH] = xs.T
        full = np.ones(NPAD, np.float32)
        full[:NSH] = dinv[s * NSH + perms[s]]
        dv = np.ascontiguousarray(full.reshape(NT, P).T)

        in_maps.append({
            "xT": xT.astype(npbf16),
            "idxw": _wrap_idx(lidx_all.reshape(-1)),
            "dstlT": np.ascontiguousarray(dstl_all.T),
            "wT": np.ascontiguousarray(w_all.T),
            "dinv": dv,
        })
    return in_maps, KA, KB


def kernel(x, edge_index, edge_weight, W1, b1, W2, b2, trace=False):
    in_maps, KA, KB = _prep_inputs(x, edge_index, edge_weight)
    shared = {
        "W1": np.asarray(W1, np.float32).astype(npbf16),
        "W2": np.asarray(W2, np.float32).astype(npbf16),
        "b1b": np.tile(np.asarray(b1, np.float32)[None, :], (P, 1)),
        "b2b": np.tile(np.asarray(b2, np.float32)[None, :], (P, 1)),
        "ident": np.eye(P, dtype=np.float32).astype(npbf16),
    }
    for im in in_maps:
        im.update(shared)

    with_bias = bool(np.any(shared["b1b"]) or np.any(shared["b2b"]))
    nc = _build_program(KA, KB, with_bias=with_bias)
    res = run_bass_kernel_spmd(nc, in_maps, core_ids=list(range(NCORES)),
                               trace=trace)
    perms = _prep_inputs.last_perms
    out = np.empty((N, OUT), np.float32)
    for s in range(NCORES):
        o = np.asarray(res.results[s]["out"], np.float32)   # [P, NT*OUT]
        o = o.reshape(P, NT, OUT).transpose(1, 0, 2).reshape(NPAD, OUT)
        out[s * NSH + perms[s]] = o[:NSH]
    kernel.last_results = res
    return out


# revision 12
# speedup vs baseline: 1.2971x; 1.0952x over previous
"""2-layer GCN (PyG GCNConv x2 + relu + log_softmax) on 8 Trainium2 NeuronCores.

Strategy: shard destination nodes (and their incoming edges) across the 8
cores. Each layer:
  1. dense  h = x @ W  data-parallel over the core's node shard,
     scaled to g = dinv * h  (dinv = 1/sqrt(weighted in-degree + self loop))
  2. AllGather the g-shards into a replicated bf16 [N, C] table
  3. edge pass: dma_gather g[src] rows (256B bf16) for the core's
     (dst-sorted, padded) edges, build bf16 selection matrices
     S^T[e, n] = w_e * (dst_local[e] == n) on the vector engine,
     segment-sum via bf16 TensorE matmuls accumulated in PSUM per
     128-node destination tile
  4. epilogue out[n] = dinv[n] * (psum[n] + g[n]) + b  (+relu / log_softmax)

dma_gather uses int16 indices, so the node table is split in two halves
(A: 25600 rows, B: 24576 rows); each dst-tile's edge list is partitioned
into A-half / B-half groups, each padded to a multiple of 128 (uniform tile
counts KA/KB across cores and dst-tiles so a single SPMD program works for
all cores). Tables and outputs use a partition-major row order
(row = p*tiles + t) so every bulk DMA moves contiguous per-partition spans.
Layer 2's 64-channel rows are zero-padded to 128 bf16 channels to satisfy
the 256B-per-descriptor gather minimum.
"""
import sys

sys.path.insert(0, "/opt/trn_rl_repo")

import numpy as np
import ml_dtypes

from concourse import bass, mybir, bacc
import concourse.tile as tile
from concourse.bass_utils import run_bass_kernel_spmd

N = 50000
E = 800000
INCH = 128
HID = 128
OUT = 64
NCORES = 8
NSH = N // NCORES          # 6250 nodes per shard
P = 128
NT = (NSH + P - 1) // P    # 49 dst tiles per shard
NPAD = NT * P              # 6272
LT = 25                    # low-half tiles
HT = NT - LT               # 24 high-half tiles
LOWN = LT * P              # 3200 low-half padded positions per shard
HIGHN = HT * P             # 3072 high-half padded positions per shard
TRA = NCORES * LOWN        # 25600 rows in table A (int16-safe)
TRB = NCORES * HIGHN       # 24576 rows in table B
CH = 32                    # gather chunk size in edge tiles

f32 = mybir.dt.float32
bf16 = mybir.dt.bfloat16
i16 = mybir.dt.int16
i32 = mybir.dt.int32
AF = mybir.ActivationFunctionType
ALU = mybir.AluOpType

npbf16 = ml_dtypes.bfloat16

_PROGRAM_CACHE = {}


def _build_program(KA, KB, trace=False, collectives=True, skip=(), with_bias=True):
    key = (KA, KB, collectives, tuple(skip), with_bias)
    if key in _PROGRAM_CACHE:
        return _PROGRAM_CACHE[key]

    TPA = NT * KA            # edge tiles in A region
    TPB = NT * KB
    TP = TPA + TPB
    EPAD = TP * P

    nc = bacc.Bacc("TRN2", target_bir_lowering=False, debug=False,
                   enable_asserts=True, num_devices=NCORES)

    # inputs
    xT_d = nc.dram_tensor("xT", [P, NPAD], bf16, kind="ExternalInput")
    idxw_d = nc.dram_tensor("idxw", [P, EPAD // 16], i16, kind="ExternalInput")
    dstl_d = nc.dram_tensor("dstlT", [P, TP], f32, kind="ExternalInput")
    wt_d = nc.dram_tensor("wT", [P, TP], f32, kind="ExternalInput")
    dinv_d = nc.dram_tensor("dinv", [P, NT], f32, kind="ExternalInput")
    w1_d = nc.dram_tensor("W1", [INCH, HID], bf16, kind="ExternalInput")
    w2_d = nc.dram_tensor("W2", [HID, OUT], bf16, kind="ExternalInput")
    b1_d = nc.dram_tensor("b1b", [P, HID], f32, kind="ExternalInput")
    b2_d = nc.dram_tensor("b2b", [P, OUT], f32, kind="ExternalInput")
    id_d = nc.dram_tensor("ident", [P, P], bf16, kind="ExternalInput")

    out_d = nc.dram_tensor("out", [P, NT * OUT], f32, kind="ExternalOutput")

    # internal DRAM (partition-major row order: row = p*LT + t  /  p*HT + t')
    g1a_d = nc.dram_tensor("g1a_shard", [LOWN, HID], bf16)
    g1b_d = nc.dram_tensor("g1b_shard", [HIGHN, HID], bf16)
    g2a_d = nc.dram_tensor("g2a_shard", [LOWN, P], bf16)
    g2b_d = nc.dram_tensor("g2b_shard", [HIGHN, P], bf16)
    t1a_d = nc.dram_tensor("table1a", [TRA, HID], bf16, addr_space="Shared")
    t1b_d = nc.dram_tensor("table1b", [TRB, HID], bf16, addr_space="Shared")
    t2a_d = nc.dram_tensor("table2a", [TRA, P], bf16, addr_space="Shared")
    t2b_d = nc.dram_tensor("table2b", [TRB, P], bf16, addr_space="Shared")

    groups = [list(range(NCORES))]

    with tile.TileContext(nc) as tc:
        with (
            tc.tile_pool(name="pers", bufs=1) as pers,
            tc.tile_pool(name="psd", bufs=2, space="PSUM") as psd,
        ):
            # persistent tiles (xT first: it gates the dense phase that
            # feeds the first AllGather)
            xT = pers.tile([P, NPAD], bf16, tag="xT")
            nc.sync.dma_start(xT[:], xT_d[:])
            idxw = pers.tile([P, EPAD // 16], i16)
            nc.sync.dma_start(idxw[:], idxw_d[:])
            dstlT = pers.tile([P, TP], f32)
            nc.sync.dma_start(dstlT[:], dstl_d[:])
            wT = pers.tile([P, TP], f32)
            nc.sync.dma_start(wT[:], wt_d[:])
            dinv = pers.tile([P, NT], f32)
            nc.sync.dma_start(dinv[:], dinv_d[:])
            w1 = pers.tile([INCH, HID], bf16)
            nc.sync.dma_start(w1[:], w1_d[:])
            w2 = pers.tile([HID, OUT], bf16)
            nc.sync.dma_start(w2[:], w2_d[:])
            if with_bias:
                b1b = pers.tile([P, HID], f32)
                nc.sync.dma_start(b1b[:], b1_d[:])
                b2b = pers.tile([P, OUT], f32)
                nc.sync.dma_start(b2b[:], b2_d[:])
            ident = pers.tile([P, P], bf16)
            nc.sync.dma_start(ident[:], id_d[:])

            iota_i = pers.tile([P, P], i32, tag="iota_i")
            nc.gpsimd.iota(iota_i[:], pattern=[[1, P]], base=0,
                           channel_multiplier=0)
            cols_b = pers.tile([P, P], bf16)
            nc.vector.tensor_copy(cols_b[:], iota_i[:])

            g1_sb = pers.tile([P, NT * HID], bf16)
            relu1T = pers.tile([P, NPAD], bf16)
            # g2 staged zero-padded to 128 channels (real 64 in low half of
            # each 128-col block); zeroed on the otherwise-idle Pool engine
            g2_sb = pers.tile([P, NT * P], bf16)
            nc.gpsimd.memset(g2_sb[:], 0.0)
            o_sb = pers.tile([P, NT * OUT], f32)
            negm_sb = pers.tile([P, NT], f32)
            se_sb = pers.tile([P, NT], f32)
            lse_sb = pers.tile([P, NT], f32)

            # ---------- phase B: dense layer 1, g1 = dinv * (x @ W1) ----------
            for t in range(NT):
                ps = psd.tile([P, HID], f32, tag="psd")
                nc.tensor.matmul(ps[:], lhsT=xT[:, t * P:(t + 1) * P],
                                 rhs=w1[:], start=True, stop=True)
                nc.vector.tensor_scalar(
                    out=g1_sb[:, t * HID:(t + 1) * HID], in0=ps[:],
                    scalar1=dinv[:, t:t + 1], scalar2=None, op0=ALU.mult)
                if t == LT - 1:
                    # low half complete -> start its AllGather now; it
                    # overlaps the high-half dense compute + B collective
                    nc.sync.dma_start(
                        g1a_d[:].rearrange("(p t) c -> p t c", p=P),
                        g1_sb[:, :LT * HID].rearrange(
                            "p (t c) -> p t c", c=HID))
                    if collectives:
                        nc.gpsimd.collective_compute(
                            "AllGather", ALU.bypass, replica_groups=groups,
                            ins=[g1a_d[:]], outs=[t1a_d[:]])
                    else:
                        nc.sync.dma_start(t1a_d[:LOWN, :], g1a_d[:])
            nc.sync.dma_start(
                g1b_d[:].rearrange("(p t) c -> p t c", p=P),
                g1_sb[:, LT * HID:].rearrange("p (t c) -> p t c", c=HID))
            if collectives:
                nc.gpsimd.collective_compute(
                    "AllGather", ALU.bypass, replica_groups=groups,
                    ins=[g1b_d[:]], outs=[t1b_d[:]])
            else:
                nc.sync.dma_start(t1b_d[:HIGHN, :], g1b_d[:])

            # ---------- edge pass over all dst tiles ----------
            def edge_pass(tabA_d, tabB_d, RW, epilogue):
                """Gather+SpMM over all dst tiles; epilogue(t, psum_tile).

                Gathered rows are always 128 bf16 channels (256B descriptors);
                the matmul consumes the first RW of them.
                """
                nchA = (NT * KA + CH - 1) // CH
                nchB = (NT * KB + CH - 1) // CH
                chunksA = [None] * nchA
                chunksB = [None] * nchB

                with (
                    tc.tile_pool(name="gbufA", bufs=3) as gpA,
                    tc.tile_pool(name="gbufB", bufs=3) as gpB,
                    tc.tile_pool(name="stbuf", bufs=4) as stp,
                    tc.tile_pool(name="pse", bufs=2, space="PSUM") as pse,
                ):
                    def chunk(region, c):
                        lst, pool, tpr, coff = (
                            (chunksA, gpA, NT * KA, 0) if region == 0 else
                            (chunksB, gpB, NT * KB, NT * KA))
                        if lst[c] is None:
                            ct = min(CH, tpr - c * CH)  # tiles in chunk
                            buf = pool.tile([P, CH * P], bf16,
                                            tag=f"g{region}")
                            if "gather" in skip:
                                lst[c] = buf
                                return buf
                            half = tabA_d[:] if region == 0 else tabB_d[:]
                            col0 = (coff + c * CH) * (P // 16)
                            nc.gpsimd.dma_gather(
                                out_ap=buf[:, :ct * P].rearrange(
                                    "p (k c) -> p k c", c=P),
                                in_ap=half,
                                idxs_ap=idxw[:, col0:col0 + ct * (P // 16)],
                                num_idxs=ct * P,
                                num_idxs_reg=ct * P,
                                elem_size=P,
                                single_packet=False,
                            )
                            lst[c] = buf
                        return lst[c]

                    for t in range(NT):
                        ps = pse.tile([P, RW], f32, tag="pse")
                        # selection matrices for this tile's A and B groups
                        stA = stp.tile([P, KA * P], bf16, tag="stA")
                        stB = stp.tile([P, KB * P], bf16, tag="stB")
                        for k in range(KA):
                            j = t * KA + k
                            if "st" in skip:
                                break
                            nc.vector.tensor_scalar(
                                out=stA[:, k * P:(k + 1) * P], in0=cols_b[:],
                                scalar1=dstlT[:, j:j + 1],
                                scalar2=wT[:, j:j + 1],
                                op0=ALU.is_equal, op1=ALU.mult)
                        for k in range(KB):
                            j = NT * KA + t * KB + k
                            if "st" in skip:
                                break
                            nc.vector.tensor_scalar(
                                out=stB[:, k * P:(k + 1) * P], in0=cols_b[:],
                                scalar1=dstlT[:, j:j + 1],
                                scalar2=wT[:, j:j + 1],
                                op0=ALU.is_equal, op1=ALU.mult)

                        for k in range(KA):
                            j = t * KA + k
                            buf = chunk(0, j // CH)
                            slot = j % CH
                            if "mm" in skip: continue
                            nc.tensor.matmul(
                                ps[:], lhsT=stA[:, k * P:(k + 1) * P],
                                rhs=buf[:, slot * P:slot * P + RW],
                                start=(k == 0),
                                stop=(KB == 0 and k == KA - 1))
                        for k in range(KB):
                            j = t * KB + k
                            buf = chunk(1, j // CH)
                            slot = j % CH
                            if "mm" in skip: continue
                            nc.tensor.matmul(
                                ps[:], lhsT=stB[:, k * P:(k + 1) * P],
                                rhs=buf[:, slot * P:slot * P + RW],
                                start=(KA == 0 and k == 0),
                                stop=(k == KB - 1))
                        epilogue(t, ps)

            # ---------- phase C: edge pass layer 1 (+ fused dense layer 2) ----
            with tc.tile_pool(name="ep1", bufs=3) as ep1, \
                 tc.tile_pool(name="pst", bufs=2, space="PSUM") as pst:
                def epi1(t, ps):
                    tmp = ep1.tile([P, HID], f32, tag="tmp1")
                    nc.vector.tensor_tensor(
                        out=tmp[:], in0=ps[:],
                        in1=g1_sb[:, t * HID:(t + 1) * HID], op=ALU.add)
                    nc.vector.tensor_scalar(
                        out=tmp[:], in0=tmp[:], scalar1=dinv[:, t:t + 1],
                        scalar2=None, op0=ALU.mult)
                    if with_bias:
                        nc.vector.tensor_tensor(
                            out=tmp[:], in0=tmp[:], in1=b1b[:], op=ALU.add)
                    relu = ep1.tile([P, HID], bf16, tag="relu1")
                    nc.scalar.activation(relu[:], tmp[:], AF.Relu)
                    pt = pst.tile([P, P], bf16, tag="pst")
                    nc.tensor.transpose(pt[:], relu[:], ident[:])
                    nc.vector.tensor_copy(relu1T[:, t * P:(t + 1) * P], pt[:])

                    # fused dense layer 2 for this tile:
                    # g2 = dinv * (relu1 @ W2), staged zero-padded to 128 ch
                    ps2 = psd.tile([P, OUT], f32, tag="psd2")
                    nc.tensor.matmul(ps2[:], lhsT=relu1T[:, t * P:(t + 1) * P],
                                     rhs=w2[:], start=True, stop=True)
                    nc.vector.tensor_scalar(
                        out=g2_sb[:, t * P:t * P + OUT], in0=ps2[:],
                        scalar1=dinv[:, t:t + 1], scalar2=None, op0=ALU.mult)
                    if t == LT - 1:
                        nc.sync.dma_start(
                            g2a_d[:].rearrange("(p t) c -> p t c", p=P),
                            g2_sb[:, :LT * P].rearrange("p (t c) -> p t c",
                                                        c=P))
                        if collectives:
                            nc.gpsimd.collective_compute(
                                "AllGather", ALU.bypass, replica_groups=groups,
                                ins=[g2a_d[:]], outs=[t2a_d[:]])
                        else:
                            nc.sync.dma_start(t2a_d[:LOWN, :], g2a_d[:])
                    if t == NT - 1:
                        nc.sync.dma_start(
                            g2b_d[:].rearrange("(p t) c -> p t c", p=P),
                            g2_sb[:, LT * P:].rearrange("p (t c) -> p t c",
                                                        c=P))
                        if collectives:
                            nc.gpsimd.collective_compute(
                                "AllGather", ALU.bypass, replica_groups=groups,
                                ins=[g2b_d[:]], outs=[t2b_d[:]])
                        else:
                            nc.sync.dma_start(t2b_d[:HIGHN, :], g2b_d[:])

                edge_pass(t1a_d, t1b_d, HID, epi1)

            # ---------- phase E: edge pass layer 2 + log_softmax ----------
            # Ln is batched over all tiles at the end (one act-table load
            # instead of alternating Exp/Ln per tile).
            with tc.tile_pool(name="ep2", bufs=3) as ep2:
                def epi2(t, ps):
                    osl = o_sb[:, t * OUT:(t + 1) * OUT]
                    nc.vector.tensor_tensor(
                        out=osl, in0=ps[:],
                        in1=g2_sb[:, t * P:t * P + OUT], op=ALU.add)
                    nc.vector.tensor_scalar(
                        out=osl, in0=osl, scalar1=dinv[:, t:t + 1],
                        scalar2=None, op0=ALU.mult)
                    if with_bias:
                        nc.vector.tensor_tensor(
                            out=osl, in0=osl, in1=b2b[:], op=ALU.add)
                    m = ep2.tile([P, 1], f32, tag="m")
                    nc.vector.tensor_reduce(
                        out=m[:], in_=osl, axis=mybir.AxisListType.X,
                        op=ALU.max)
                    nc.vector.tensor_scalar(
                        out=negm_sb[:, t:t + 1], in0=m[:], scalar1=-1.0,
                        scalar2=None, op0=ALU.mult)
                    ex = ep2.tile([P, OUT], f32, tag="ex")
                    nc.scalar.activation(ex[:], osl, AF.Exp,
                                         bias=negm_sb[:, t:t + 1],
                                         accum_out=se_sb[:, t:t + 1])
                    # finalize + write out in two halves so the low half's
                    # log_softmax and DMA overlap the high half's edge pass
                    if t in (LT - 1, NT - 1):
                        lo = 0 if t == LT - 1 else LT
                        hi = t + 1
                        nc.scalar.activation(lse_sb[:, lo:hi],
                                             se_sb[:, lo:hi], AF.Ln)
                        for u in range(lo, hi):
                            nc.vector.tensor_scalar(
                                out=o_sb[:, u * OUT:(u + 1) * OUT],
                                in0=o_sb[:, u * OUT:(u + 1) * OUT],
                                scalar1=negm_sb[:, u:u + 1],
                                scalar2=lse_sb[:, u:u + 1],
                                op0=ALU.add, op1=ALU.subtract)
                        nc.sync.dma_start(out_d[:, lo * OUT:hi * OUT],
                                          o_sb[:, lo * OUT:hi * OUT])

                edge_pass(t2a_d, t2b_d, OUT, epi2)

    nc.compile()
    _PROGRAM_CACHE[key] = nc
    return nc


def _wrap_idx(lidx):
    """[EPAD] int -> [128, EPAD//16] int16 (16-partition wrap, 8x replicated)."""
    n = lidx.shape[0]
    w16 = lidx.reshape(n // 16, 16).T.astype(np.int16)   # [16, n/16]
    return np.ascontiguousarray(np.tile(w16, (8, 1)))


def _balance(aA, aB):
    """Assign a shard's nodes to tiles (low nodes -> tiles [0,LT), high ->
    [LT,NT)) so that per-tile A-half and B-half in-edge counts stay under
    KA*128 / KB*128 with KA=9, KB=8 if possible.  Returns perm (node order:
    position t*128+p holds node perm[t*128+p])."""
    tot = aA + aB
    capA_t = 9 * P    # soft targets
    capB_t = 8 * P
    buckets = [[] for _ in range(NT)]
    for lo, hi, t0, t1 in ((0, LOWN, 0, LT), (LOWN, NSH, LT, NT)):
        nodes = np.arange(lo, hi)
        order = nodes[np.argsort(-tot[lo:hi], kind="stable")]
        tiles = np.arange(t0, t1)
        ntile = t1 - t0
        cap = np.full(ntile, P, np.int64)
        if t1 == NT:
            cap[ntile - 1] = NSH - lo - (ntile - 1) * P
        loadA = np.zeros(ntile)
        loadB = np.zeros(ntile)
        for nd in order:
            a, b = aA[nd], aB[nd]
            score = np.maximum((loadA + a) / capA_t, (loadB + b) / capB_t)
            score = np.where(cap > 0, score, np.inf)
            ti = int(np.argmin(score))
            buckets[tiles[ti]].append(nd)
            cap[ti] -= 1
            loadA[ti] += a
            loadB[ti] += b
        # repair pass: swap nodes between tiles until no tile exceeds the
        # KA/KB targets on either half (usually a handful of swaps)
        for _ in range(400):
            over = np.maximum(loadA - capA_t, 0) + np.maximum(loadB - capB_t, 0)
            u = int(np.argmax(over))
            if over[u] <= 0:
                break
            overA = loadA[u] > capA_t
            la, lb = (aA, aB) if overA else (aB, aA)
            lu, lv_all = (loadA, loadB) if overA else (loadB, loadA)
            best = None
            bu = buckets[tiles[u]]
            for vi in np.argsort(lu):
                v = int(vi)
                if v == u:
                    continue
                bv = buckets[tiles[v]]
                # swap nd1 (from u) with nd2 (from v): want big la drop on u
                # without pushing v over either cap
                n1 = max(bu, key=lambda nd: la[nd])
                n2 = min(bv, key=lambda nd: la[nd])
                d = la[n1] - la[n2]
                if d <= 0:
                    continue
                if lu[v] + d > (capA_t if overA else capB_t):
                    continue
                if lv_all[v] + (lb[n1] - lb[n2]) > (capB_t if overA else capA_t):
                    continue
                best = (v, n1, n2)
                break
            if best is None:
                break
            v, n1, n2 = best
            bu[bu.index(n1)] = n2
            buckets[tiles[v]][buckets[tiles[v]].index(n2)] = n1
            loadA[u] += aA[n2] - aA[n1]
            loadB[u] += aB[n2] - aB[n1]
            loadA[v] += aA[n1] - aA[n2]
            loadB[v] += aB[n1] - aB[n2]
    return np.concatenate([np.asarray(b, np.int64) for b in buckets])


def _prep_inputs(x, edge_index, edge_weight):
    src = np.asarray(edge_index[0], dtype=np.int64)
    dst = np.asarray(edge_index[1], dtype=np.int64)
    w = np.asarray(edge_weight, dtype=np.float32)
    x = np.asarray(x, dtype=np.float32)

    deg = np.bincount(dst, weights=w.astype(np.float64), minlength=N)
    deg = deg.astype(np.float32) + 1.0
    dinv = (1.0 / np.sqrt(deg)).astype(np.float32)

    shard_src = src // NSH
    shard_dst = dst // NSH
    # half = src's ORIGINAL local id >= LOWN; the balancer below preserves
    # half membership (low nodes only land in permuted positions < LOWN),
    # so this is permutation-independent.
    halfe = ((src % NSH) >= LOWN).astype(np.int64)

    perms = []
    iperms = np.empty((NCORES, NSH), np.int64)
    for s in range(NCORES):
        m = shard_dst == s
        dl = dst[m] - s * NSH
        he = halfe[m]
        aA = np.bincount(dl[he == 0], minlength=NSH).astype(np.int64)
        aB = np.bincount(dl[he == 1], minlength=NSH).astype(np.int64)
        perm = _balance(aA, aB)
        perms.append(perm)
        iperms[s][perm] = np.arange(NSH)
    _prep_inputs.last_perms = perms

    per_core = []
    for s in range(NCORES):
        m = shard_dst == s
        es = src[m]
        ew = w[m]
        half = halfe[m]
        edp = iperms[s][dst[m] - s * NSH]      # permuted local dst position
        t = edp >> 7
        pos = iperms[shard_src[m], es % NSH]   # permuted position of src
        pt = pos >> 7
        pp = pos & 127
        # table row within its half-table, partition-major:
        # A: row = p*LT + t (t<LT);  B: row = p*HT + (t-LT)
        trow = np.where(pos < LOWN,
                        shard_src[m] * LOWN + pp * LT + pt,
                        shard_src[m] * HIGHN + pp * HT + (pt - LT))
        key = (half * NT + t)
        order = np.argsort(key, kind="stable")
        per_core.append((trow[order], edp[order], ew[order],
                         t[order], half[order]))

    # group counts -> uniform KA/KB
    cntA = np.zeros((NCORES, NT), np.int64)
    cntB = np.zeros((NCORES, NT), np.int64)
    for s in range(NCORES):
        es, ed, ew, t, half = per_core[s]
        for h, cnt in ((0, cntA), (1, cntB)):
            hm = half == h
            cnt[s] = np.bincount(t[hm], minlength=NT)
    KA = int(np.ceil(cntA.max() / P))
    KB = int(np.ceil(cntB.max() / P))

    in_maps = []
    for s in range(NCORES):
        es, ed, ew, t, half = per_core[s]
        lidx_all = np.zeros((NT * (KA + KB), P), np.int64)
        dstl_all = np.zeros((NT * (KA + KB), P), np.float32)
        w_all = np.zeros((NT * (KA + KB), P), np.float32)
        for h, K, base_tile in ((0, KA, 0), (1, KB, NT * KA)):
            hm = half == h
            eh, edh, ewh, th = es[hm], ed[hm], ew[hm], t[hm]
            for tt in range(NT):
                tm = th == tt
                cnt = int(tm.sum())
                row = base_tile + tt * K
                flat_l = np.zeros(K * P, np.int64)
                flat_d = np.zeros(K * P, np.float32)
                flat_w = np.zeros(K * P, np.float32)
                flat_l[:cnt] = eh[tm]
                flat_d[:cnt] = (edh[tm] & 127).astype(np.float32)
                flat_w[:cnt] = ewh[tm]
                lidx_all[row:row + K] = flat_l.reshape(K, P)
                dstl_all[row:row + K] = flat_d.reshape(K, P)
                w_all[row:row + K] = flat_w.reshape(K, P)

        xs = x[s * NSH + perms[s]]
        xT = np.zeros((P, NPAD), np.float32)
        xT[:, :NSH] = xs.T
        full = np.ones(NPAD, np.float32)
        full[:NSH] = dinv[s * NSH + perms[s]]
        dv = np.ascontiguousarray(full.reshape(NT, P).T)

        in_maps.append({
            "xT": xT.astype(npbf16),
            "idxw": _wrap_idx(lidx_all.reshape(-1)),
            "dstlT": np.ascontiguousarray(dstl_all.T),
            "wT": np.ascontiguousarray(w_all.T),
            "dinv": dv,
        })
    return in_maps, KA, KB


def kernel(x, edge_index, edge_weight, W1, b1, W2, b2, trace=False):
    in_maps, KA, KB = _prep_inputs(x, edge_index, edge_weight)
    shared = {
        "W1": np.asarray(W1, np.float32).astype(npbf16),
        "W2": np.asarray(W2, np.float32).astype(npbf16),
        "b1b": np.tile(np.asarray(b1, np.float32)[None, :], (P, 1)),
        "b2b": np.tile(np.asarray(b2, np.float32)[None, :], (P, 1)),
        "ident": np.eye(P, dtype=np.float32).astype(npbf16),
    }
    for im in in_maps:
        im.update(shared)

    with_bias = bool(np.any(shared["b1b"]) or np.any(shared["b2b"]))
    nc = _build_program(KA, KB, with_bias=with_bias)
    res = run_bass_kernel_spmd(nc, in_maps, core_ids=list(range(NCORES)),
                               trace=trace)
    perms = _prep_inputs.last_perms
    out = np.empty((N, OUT), np.float32)
    for s in range(NCORES):
        o = np.asarray(res.results[s]["out"], np.float32)   # [P, NT*OUT]
        o = o.reshape(P, NT, OUT).transpose(1, 0, 2).reshape(NPAD, OUT)
        out[s * NSH + perms[s]] = o[:NSH]
    kernel.last_results = res
    return out
